# revision 27
# baseline (speedup 1.0000x reference)
"""Segment-mean (scatter_mean over sorted index) on Trainium2, 8 NeuronCores.

Strategy
--------
index is sorted, so segment s's edges are a contiguous row-range of x.
The output is processed in windows of P=128 segments; window g draws from
a contiguous edge-slice of x. The host:
  * pre-scales every edge row by 1/clamp(count[seg],1)  (so segment-MEAN
    becomes plain segment-SUM on device),
  * splits the scaled rows into a bf16 hi/lo pair (hi = bf16(x),
    lo = bf16(x - hi); hi+lo carries ~17 mantissa bits, inside the fp32
    envelope, while enabling full-rate bf16 matmuls),
  * assigns windows to the 8 cores balanced by their edge-tile counts and
    pads each rank-w window to a common tile count B_w (max over cores),
    so the device program is fully static and identical across cores —
    all data-dependence lives in the per-core input tensors,
  * lays the gathered slabs out partition-major per load-group of GG
    windows, so each ~1.8 MB load expands to just 128 multi-KB DMA
    descriptors (the HWDGE descriptor generator is a serial resource).

On device, per window (B = B_w edge tiles of 128 edges):
  * ONE batched DVE is_equal builds the bf16 one-hot [128, B*128]:
    onehot[e, j*128+s] = (rel[e, j] == s), via an iota tile and a step-0
    broadcast AP of the per-tile relative indices
  * B matmuls accumulate psum[s, 0:128] += oh_j^T @ hi_j and
    psum[s, 128:256] += oh_j^T @ lo_j in one [K=128, N=256] pass each
  * ACT copies the hi|lo pair out of PSUM, the DVE adds the halves,
    and the store fires on ACT's DGE to static output rows.

Padding edges carry rel = -1 -> all-zero one-hot column -> no
contribution. Empty segments have sum 0 -> output 0 (count clamped on
host).
"""

import numpy as np
import ml_dtypes

import concourse.bacc as bacc
import concourse.mybir as mybir
import concourse.tile as tile
from concourse.bass_utils import run_bass_kernel_spmd

P = 128
D = 128
NCORES = 8
GG = 4  # windows per load-group
BF16 = ml_dtypes.bfloat16

_nc_cache: dict = {}


def _groups(Bs):
    """Split window ranks into load-groups of GG."""
    out = []
    for w0 in range(0, len(Bs), GG):
        out.append(list(range(w0, min(w0 + GG, len(Bs)))))
    return out


def _build(Bs: tuple):
    """Compile the per-core SPMD program. Bs[w] = edge tiles in window w."""
    if Bs in _nc_cache:
        return _nc_cache[Bs]

    f32 = mybir.dt.float32
    bf16 = mybir.dt.bfloat16
    WN = len(Bs)
    T = int(sum(Bs))
    Bmax = int(max(Bs))
    offs = np.concatenate([[0], np.cumsum(Bs)]).astype(int)
    groups = _groups(Bs)
    gmax = max(sum(Bs[w] for w in g) for g in groups)

    nc = bacc.Bacc("TRN2", target_bir_lowering=False, debug=False,
                   num_devices=NCORES)
    fp8 = mybir.dt.float8e4
    xhl_d = nc.dram_tensor("xhl", [T * P, 2 * D], bf16,
                           kind="ExternalInput").ap()
    ohx_d = nc.dram_tensor("ohx", [T * P, P], fp8,
                           kind="ExternalInput").ap()
    out_d = nc.dram_tensor("out", [WN * P, D], f32, kind="ExternalOutput").ap()

    with tile.TileContext(nc) as tc:
        with (
            tc.tile_pool(name="const", bufs=1) as cpool,
            tc.tile_pool(name="xin", bufs=3) as xpool,
            tc.tile_pool(name="oh", bufs=3) as ohpool,
            tc.tile_pool(name="res", bufs=6) as rpool,
            tc.tile_pool(name="ps", bufs=7, space="PSUM") as pspool,
            tc.tile_pool(name="pswarm", bufs=1, space="PSUM") as wpool,
        ):
            # ~5us of dummy matmuls to flip the PE HAM clock-gate to 8/8
            # before the first real accumulation group arrives.
            wsrc = cpool.tile([P, 2 * D], bf16)
            nc.vector.memset(wsrc[:], 0)
            warm = wpool.tile([P, 2 * D], f32)
            for _ in range(24):
                nc.tensor.matmul(out=warm[:], lhsT=wsrc[:, :P],
                                 rhs=wsrc[:], start=True, stop=True)

            def extract(w, ps):
                """hi+lo merge: ACT pair-copy out of PSUM, DVE SBUF add."""
                res = rpool.tile([P, D], f32, tag="res")
                nc.vector.tensor_reduce(
                    out=res[:],
                    in_=ps[:].rearrange("p (k d) -> p d k", k=2),
                    axis=mybir.AxisListType.X, op=mybir.AluOpType.add)
                nc.scalar.dma_start(out=out_d[P * w:P * (w + 1), :],
                                    in_=res[:])

            pending = []  # extractions deferred 4 windows so the DVE
            # reduce never head-of-line-blocks upcoming one-hot builds
            for g in groups:
                t0 = int(offs[g[0]])
                Bt = int(sum(Bs[w] for w in g))
                xw = xpool.tile([P, gmax * 2 * D], bf16, tag="xw")
                # partition-major slab: DRAM row t0*P + p*Bt + jj.
                # One DMA per window (into slices of the group tile) so each
                # window's data lands as early as possible and the per-DMA
                # completion receipt amortizes per window, not per group.
                src_g = xhl_d[t0 * P:(t0 + Bt) * P, :].rearrange(
                    "(p j) c -> p j c", j=Bt)
                soh_g = ohx_d[t0 * P:(t0 + Bt) * P, :].rearrange(
                    "(p j) c -> p j c", j=Bt)
                ohg = ohpool.tile([P, gmax * P], fp8, tag="oh")
                jo2 = 0
                for w in g:
                    B = int(Bs[w])
                    nc.sync.dma_start(
                        out=xw[:, jo2 * 2 * D:(jo2 + B) * 2 * D].rearrange(
                            "p (j c) -> p j c", c=2 * D),
                        in_=src_g[:, jo2:jo2 + B, :])
                    nc.sync.dma_start(
                        out=ohg[:, jo2 * P:(jo2 + B) * P].rearrange(
                            "p (j c) -> p j c", c=P),
                        in_=soh_g[:, jo2:jo2 + B, :])
                    jo2 += B
                jo = 0
                for w in g:
                    B = int(Bs[w])
                    ps = pspool.tile([P, 2 * D], f32, tag="ps")
                    for j in range(B):
                        c0 = (jo + j) * 2 * D
                        nc.tensor.matmul(out=ps[:],
                                         lhsT=ohg[:, (jo + j) * P:
                                                  (jo + j + 1) * P],
                                         rhs=xw[:, c0:c0 + 2 * D],
                                         start=(j == 0), stop=(j == B - 1))
                    pending.append((w, ps))
                    if len(pending) > 4:
                        extract(*pending.pop(0))
                    jo += B
            for it in pending:
                extract(*it)

    nc.compile()
    _nc_cache[Bs] = nc
    return nc


def _prepare(x: np.ndarray, index: np.ndarray, n_segments: int):
    """Host-side shard/gather prep.

    Returns (Bs, in_maps, asg) where asg[m][w] = global window id of core
    m's rank-w slot (or -1 for a dummy), for output reassembly.
    """
    E, d = x.shape
    assert d == D
    idx = np.asarray(index).astype(np.int64).ravel()

    if np.any(idx[1:] < idx[:-1]):  # tolerate unsorted input
        perm = np.argsort(idx, kind="stable")
        idx = idx[perm]
        x = x[perm]

    G = -(-n_segments // P)  # global 128-segment windows
    bounds = np.searchsorted(idx, np.arange(0, (G + 1) * P, P)).astype(np.int64)
    wcnt = bounds[1:] - bounds[:-1]
    wtiles = np.maximum(1, -(-wcnt // P))  # >=1 so every window is scheduled

    # Balance windows across cores by tile count (greedy, desc).
    order = np.argsort(-wtiles, kind="stable")
    loads = np.zeros(NCORES, np.int64)
    per_core: list[list[int]] = [[] for _ in range(NCORES)]
    for g in order:
        m = int(np.argmin(loads))
        per_core[m].append(int(g))
        loads[m] += wtiles[g]
    WN = max(len(c) for c in per_core)
    for m in range(NCORES):
        per_core[m] += [-1] * (WN - len(per_core[m]))
    asg = np.array(per_core)                          # [NCORES, WN]
    tl = np.where(asg >= 0, wtiles[np.maximum(asg, 0)], 1)
    Bs = tuple(int(b) for b in tl.max(axis=0))        # common schedule
    T = sum(Bs)
    offs = np.concatenate([[0], np.cumsum(Bs)]).astype(np.int64)

    # permutation: tile-major (t*P + p) -> partition-major per load-group
    perm2 = np.empty(T * P, np.int64)
    for g in _groups(Bs):
        t0, Bt = int(offs[g[0]]), int(sum(Bs[w] for w in g))
        tt, pp = np.meshgrid(np.arange(Bt), np.arange(P), indexing='ij')
        # position t0*P + p*Bt + tt  holds  tile-major element (t0+tt)*P + p
        perm2[t0 * P + (pp * Bt + tt).ravel()] = ((t0 + tt) * P + pp).ravel()

    # Pre-scale by 1/count and split to bf16 hi/lo.
    cnt = np.bincount(idx, minlength=n_segments).astype(np.float32)
    inv = (1.0 / np.maximum(cnt, 1.0)).astype(np.float32)
    xs = x * inv[idx][:, None]
    hi = xs.astype(BF16)
    lo = (xs - hi.astype(np.float32)).astype(BF16)

    import concourse.mybir as _mb
    FP8 = _mb.dt.np(_mb.dt.float8e4)
    lut = np.zeros((P + 1, P), FP8)
    lut[np.arange(P), np.arange(P)] = FP8(1.0)
    in_maps = []
    for m in range(NCORES):
        gi = np.zeros(T * P, np.int64)
        rel = np.full(T * P, -1.0, np.float32)
        for w in range(WN):
            g = asg[m, w]
            if g < 0:
                continue
            s0, c = bounds[g], int(wcnt[g])
            B = Bs[w]
            o = int(offs[w]) * P
            k = np.arange(B * P)
            rows = s0 + np.minimum(k, max(c - 1, 0))
            np.clip(rows, 0, E - 1, out=rows)
            gi[o:o + B * P] = rows
            valid = k < c
            rel[o:o + B * P] = np.where(valid, (idx[rows] - g * P), -1)
        gi2 = gi[perm2]
        xhl = np.empty((T * P, 2 * D), BF16)
        xhl[:, :D] = hi[gi2]
        xhl[:, D:] = lo[gi2]
        rel2 = rel[perm2].astype(np.int64)
        ohx = lut[np.where(rel2 >= 0, rel2, P)]
        in_maps.append({"xhl": xhl, "ohx": ohx})
    return Bs, in_maps, asg


def kernel_with_results(x, index, dim_size, **run_kwargs):
    x = np.ascontiguousarray(np.asarray(x, dtype=np.float32))
    n = int(np.asarray(dim_size))
    Bs, in_maps, asg = _prepare(x, np.asarray(index), n)
    nc = _build(Bs)
    r = None
    for attempt in range(3):  # the device occasionally wedges transiently
        try:
            r = run_bass_kernel_spmd(nc, in_maps,
                                     core_ids=list(range(NCORES)),
                                     **run_kwargs)
            break
        except Exception:
            if attempt == 2:
                raise
            import time
            time.sleep(5.0)
    G = -(-n // P)
    out = np.zeros((G * P, D), np.float32)
    for m in range(NCORES):
        om = r.results[m]["out"]
        for w in range(asg.shape[1]):
            g = asg[m, w]
            if g >= 0:
                out[g * P:(g + 1) * P] = om[w * P:(w + 1) * P]
    return np.ascontiguousarray(out[:n]), r


def kernel(x, index, dim_size):
    out, _ = kernel_with_results(x, index, dim_size)
    return out


# revision 28
# speedup vs baseline: 1.4196x; 1.4196x over previous
"""Segment-mean (scatter_mean over sorted index) on Trainium2, 8 NeuronCores.

Strategy
--------
index is sorted, so segment s's edges are a contiguous row-range of x.
The output is processed in windows of P=128 segments; window g draws from
a contiguous edge-slice of x. The host:
  * pre-scales every edge row by 1/clamp(count[seg],1)  (so segment-MEAN
    becomes plain segment-SUM on device),
  * splits the scaled rows into a bf16 hi/lo pair (hi = bf16(x),
    lo = bf16(x - hi); hi+lo carries ~17 mantissa bits, inside the fp32
    envelope, while enabling full-rate bf16 matmuls),
  * assigns windows to the 8 cores balanced by their edge-tile counts and
    pads each rank-w window to a common tile count B_w (max over cores),
    so the device program is fully static and identical across cores —
    all data-dependence lives in the per-core input tensors,
  * lays the gathered slabs out partition-major per load-group of GG
    windows, so each ~1.8 MB load expands to just 128 multi-KB DMA
    descriptors (the HWDGE descriptor generator is a serial resource).

On device, per window (B = B_w edge tiles of 128 edges):
  * ONE batched DVE is_equal builds the bf16 one-hot [128, B*128]:
    onehot[e, j*128+s] = (rel[e, j] == s), via an iota tile and a step-0
    broadcast AP of the per-tile relative indices
  * B matmuls accumulate psum[s, 0:128] += oh_j^T @ hi_j and
    psum[s, 128:256] += oh_j^T @ lo_j in one [K=128, N=256] pass each
  * ACT copies the hi|lo pair out of PSUM, the DVE adds the halves,
    and the store fires on ACT's DGE to static output rows.

Padding edges carry rel = -1 -> all-zero one-hot column -> no
contribution. Empty segments have sum 0 -> output 0 (count clamped on
host).
"""

import numpy as np
import ml_dtypes

import concourse.bacc as bacc
import concourse.mybir as mybir
import concourse.tile as tile
from concourse.bass_utils import run_bass_kernel_spmd

P = 128
D = 128
NCORES = 8
GG = 4  # windows per load-group
BF16 = ml_dtypes.bfloat16

_nc_cache: dict = {}


def _groups(Bs):
    """Split window ranks into load-groups of GG."""
    out = []
    for w0 in range(0, len(Bs), GG):
        out.append(list(range(w0, min(w0 + GG, len(Bs)))))
    return out


def _build(Bs: tuple):
    """Compile the per-core SPMD program. Bs[w] = edge tiles in window w."""
    if Bs in _nc_cache:
        return _nc_cache[Bs]

    f32 = mybir.dt.float32
    bf16 = mybir.dt.bfloat16
    WN = len(Bs)
    T = int(sum(Bs))
    Bmax = int(max(Bs))
    offs = np.concatenate([[0], np.cumsum(Bs)]).astype(int)
    groups = _groups(Bs)
    gmax = max(sum(Bs[w] for w in g) for g in groups)

    nc = bacc.Bacc("TRN2", target_bir_lowering=False, debug=False,
                   num_devices=NCORES)
    xhl_d = nc.dram_tensor("xhl", [T * P, 2 * D], bf16,
                           kind="ExternalInput").ap()
    rel_d = nc.dram_tensor("rel", [P, T], bf16, kind="ExternalInput").ap()
    iota_d = nc.dram_tensor("iota", [P, Bmax * P], bf16,
                            kind="ExternalInput").ap()
    out_d = nc.dram_tensor("out", [WN * P, D], f32, kind="ExternalOutput").ap()

    with tile.TileContext(nc) as tc:
        with (
            tc.tile_pool(name="const", bufs=1) as cpool,
            tc.tile_pool(name="xin", bufs=3) as xpool,
            tc.tile_pool(name="oh", bufs=8) as ohpool,
            tc.tile_pool(name="pair", bufs=5) as ppool,
            tc.tile_pool(name="res", bufs=6) as rpool,
            tc.tile_pool(name="ps", bufs=7, space="PSUM") as pspool,
            tc.tile_pool(name="pswarm", bufs=1, space="PSUM") as wpool,
        ):
            iota_t = cpool.tile([P, Bmax * P], bf16)
            nc.sync.dma_start(out=iota_t[:], in_=iota_d[:])
            rel_t = cpool.tile([P, T], bf16)
            nc.sync.dma_start(out=rel_t[:], in_=rel_d[:])

            # ~5us of dummy matmuls to flip the PE HAM clock-gate to 8/8
            # before the first real accumulation group arrives.
            warm = wpool.tile([P, 2 * D], f32)
            for _ in range(24):
                nc.tensor.matmul(out=warm[:], lhsT=iota_t[:, :P],
                                 rhs=iota_t[:, :2 * D], start=True, stop=True)

            def extract(w, ps):
                """hi+lo merge: ACT pair-copy out of PSUM, DVE SBUF add."""
                res = rpool.tile([P, D], f32, tag="res")
                pair = ppool.tile([P, 2 * D], f32, tag="pair")
                nc.scalar.copy(out=pair[:], in_=ps[:])
                nc.vector.tensor_add(out=res[:], in0=pair[:, :D],
                                     in1=pair[:, D:2 * D])
                nc.scalar.dma_start(out=out_d[P * w:P * (w + 1), :],
                                    in_=res[:])

            pending = []  # extractions deferred 4 windows so the DVE
            # reduce never head-of-line-blocks upcoming one-hot builds
            for g in groups:
                t0 = int(offs[g[0]])
                Bt = int(sum(Bs[w] for w in g))
                xw = xpool.tile([P, gmax * 2 * D], bf16, tag="xw")
                # partition-major slab: DRAM row t0*P + p*Bt + jj.
                # One DMA per window (into slices of the group tile) so each
                # window's data lands as early as possible and the per-DMA
                # completion receipt amortizes per window, not per group.
                src_g = xhl_d[t0 * P:(t0 + Bt) * P, :].rearrange(
                    "(p j) c -> p j c", j=Bt)
                jo2 = 0
                for w in g:
                    B = int(Bs[w])
                    nc.sync.dma_start(
                        out=xw[:, jo2 * 2 * D:(jo2 + B) * 2 * D].rearrange(
                            "p (j c) -> p j c", c=2 * D),
                        in_=src_g[:, jo2:jo2 + B, :])
                    jo2 += B
                jo = 0
                for w in g:
                    B = int(Bs[w])
                    ow = int(offs[w])
                    oh = ohpool.tile([P, Bmax * P], bf16, tag="oh")
                    nc.vector.tensor_tensor(
                        out=oh[:, :B * P].rearrange("p (j s) -> p j s", s=P),
                        in0=iota_t[:, :B * P].rearrange("p (j s) -> p j s",
                                                        s=P),
                        in1=rel_t[:, ow:ow + B].to_broadcast([P, B, P]),
                        op=mybir.AluOpType.is_equal)
                    ps = pspool.tile([P, 2 * D], f32, tag="ps")
                    for j in range(B):
                        c0 = (jo + j) * 2 * D
                        nc.tensor.matmul(out=ps[:],
                                         lhsT=oh[:, j * P:(j + 1) * P],
                                         rhs=xw[:, c0:c0 + 2 * D],
                                         start=(j == 0), stop=(j == B - 1))
                    pending.append((w, ps))
                    if len(pending) > 4:
                        extract(*pending.pop(0))
                    jo += B
            for it in pending:
                extract(*it)

    nc.compile()
    _nc_cache[Bs] = nc
    return nc


def _prepare(x: np.ndarray, index: np.ndarray, n_segments: int):
    """Host-side shard/gather prep.

    Returns (Bs, in_maps, asg) where asg[m][w] = global window id of core
    m's rank-w slot (or -1 for a dummy), for output reassembly.
    """
    E, d = x.shape
    assert d == D
    idx = np.asarray(index).astype(np.int64).ravel()

    if np.any(idx[1:] < idx[:-1]):  # tolerate unsorted input
        perm = np.argsort(idx, kind="stable")
        idx = idx[perm]
        x = x[perm]

    G = -(-n_segments // P)  # global 128-segment windows
    bounds = np.searchsorted(idx, np.arange(0, (G + 1) * P, P)).astype(np.int64)
    wcnt = bounds[1:] - bounds[:-1]
    wtiles = np.maximum(1, -(-wcnt // P))  # >=1 so every window is scheduled

    # Balance windows across cores by tile count (greedy, desc).
    order = np.argsort(-wtiles, kind="stable")
    loads = np.zeros(NCORES, np.int64)
    per_core: list[list[int]] = [[] for _ in range(NCORES)]
    for g in order:
        m = int(np.argmin(loads))
        per_core[m].append(int(g))
        loads[m] += wtiles[g]
    WN = max(len(c) for c in per_core)
    for m in range(NCORES):
        per_core[m] += [-1] * (WN - len(per_core[m]))
    asg = np.array(per_core)                          # [NCORES, WN]
    tl = np.where(asg >= 0, wtiles[np.maximum(asg, 0)], 1)
    Bs = tuple(int(b) for b in tl.max(axis=0))        # common schedule
    T = sum(Bs)
    offs = np.concatenate([[0], np.cumsum(Bs)]).astype(np.int64)

    # permutation: tile-major (t*P + p) -> partition-major per load-group
    perm2 = np.empty(T * P, np.int64)
    for g in _groups(Bs):
        t0, Bt = int(offs[g[0]]), int(sum(Bs[w] for w in g))
        tt, pp = np.meshgrid(np.arange(Bt), np.arange(P), indexing='ij')
        # position t0*P + p*Bt + tt  holds  tile-major element (t0+tt)*P + p
        perm2[t0 * P + (pp * Bt + tt).ravel()] = ((t0 + tt) * P + pp).ravel()

    # Pre-scale by 1/count and split to bf16 hi/lo.
    cnt = np.bincount(idx, minlength=n_segments).astype(np.float32)
    inv = (1.0 / np.maximum(cnt, 1.0)).astype(np.float32)
    xs = x * inv[idx][:, None]
    hi = xs.astype(BF16)
    lo = (xs - hi.astype(np.float32)).astype(BF16)

    in_maps = []
    iota = np.ascontiguousarray(np.broadcast_to(
        np.arange(max(Bs) * P, dtype=np.float32) % P,
        (P, max(Bs) * P))).astype(BF16)
    for m in range(NCORES):
        gi = np.zeros(T * P, np.int64)
        rel = np.full(T * P, -1.0, np.float32)
        for w in range(WN):
            g = asg[m, w]
            if g < 0:
                continue
            s0, c = bounds[g], int(wcnt[g])
            B = Bs[w]
            o = int(offs[w]) * P
            k = np.arange(B * P)
            rows = s0 + np.minimum(k, max(c - 1, 0))
            np.clip(rows, 0, E - 1, out=rows)
            gi[o:o + B * P] = rows
            valid = k < c
            rel[o:o + B * P] = np.where(valid, (idx[rows] - g * P), -1)
        gi2 = gi[perm2]
        xhl = np.empty((T * P, 2 * D), BF16)
        xhl[:, :D] = hi[gi2]
        xhl[:, D:] = lo[gi2]
        in_maps.append({
            "xhl": xhl,
            "rel": np.ascontiguousarray(rel.reshape(T, P).T.astype(BF16)),
            "iota": iota,
        })
    return Bs, in_maps, asg


def kernel_with_results(x, index, dim_size, **run_kwargs):
    x = np.ascontiguousarray(np.asarray(x, dtype=np.float32))
    n = int(np.asarray(dim_size))
    Bs, in_maps, asg = _prepare(x, np.asarray(index), n)
    nc = _build(Bs)
    r = None
    for attempt in range(3):  # the device occasionally wedges transiently
        try:
            r = run_bass_kernel_spmd(nc, in_maps,
                                     core_ids=list(range(NCORES)),
                                     **run_kwargs)
            break
        except Exception:
            if attempt == 2:
                raise
            import time
            time.sleep(5.0)
    G = -(-n // P)
    out = np.zeros((G * P, D), np.float32)
    for m in range(NCORES):
        om = r.results[m]["out"]
        for w in range(asg.shape[1]):
            g = asg[m, w]
            if g >= 0:
                out[g * P:(g + 1) * P] = om[w * P:(w + 1) * P]
    return np.ascontiguousarray(out[:n]), r


def kernel(x, index, dim_size):
    out, _ = kernel_with_results(x, index, dim_size)
    return out
